# revision 30
# baseline (speedup 1.0000x reference)
"""Causal multi-head attention + RoPE — Trainium2 Bass kernel, 8-core SPMD.

Sharding: batch (2) x head-groups (4 heads each) -> 8 cores.  Wq/Wk/Wv
column-sharded per head group, Wo row-sharded; each core computes a
partial out-projection [S, D] and the host sums the 4 partials per
batch.

Hybrid precision, chosen from a numpy noise study of the 2e-2
max-error budget: the VALUE path (x, Wv, V, O^T, Wo and the Q/K
projections) runs in fp16 — fp8's ~2-4% element noise propagates ~1:1
through contractions and busts the budget — while the exp'd rotated
Q/K tiles are stored fp8 and the score matmuls use fp8 DoubleRow
(2 k-subtiles per matmul at 0.5 cycles/row), which softmax
normalization and attention averaging absorb (measured 1.3e-2 total).

Structure:
  - scores/attn@V work on transposed S^T[k,q] tiles; RoPE writes fp8
    q/k directly in DoubleRow score layout (head h on partition rows
    32h..32h+31, [re|ro] subtiles), so no regrouping pass exists
  - one Exp per (head, k-block-pair) over a 2-bank [128, 2, 512-off]
    PSUM window with scale=1/8, bias=-3 (range-safe, shift-invariant)
  - causal masking via Pool (gpsimd) multiplies on the exp'd at tiles
  - denominators ride as a ones-column in V; softmax division is
    reciprocal -> SBUF->SBUF DMA partition-broadcast -> one DVE
    multiply (PE fp32r broadcast on the final pair to cut tail latency)
  - out-projection per 128-row s-block, PSUM -> SBUF copy -> DMA, with
    the last q-range's copies on ScalarE (idle at the tail)
  - projections/RoPE for q-range n+1 run as fillers inside q-range n's
    attention; ScalarE exp stream and the PE overlap throughout
"""

import os
import sys
from contextlib import ExitStack

import numpy as np

for _p in ("/opt/trn_rl_repo", "/root/.axon_site/_ro/trn_rl_repo"):
    if os.path.isdir(_p) and _p not in sys.path:
        sys.path.insert(0, _p)

import ml_dtypes  # noqa: E402
import concourse.bass as bass  # noqa: E402
import concourse.tile as tile  # noqa: E402
from concourse import bacc, mybir  # noqa: E402
from concourse.bass_utils import run_bass_kernel_spmd  # noqa: E402

F16 = mybir.dt.float16
FP8 = mybir.dt.float8e4
DR = mybir.MatmulPerfMode.DoubleRow
F32 = mybir.dt.float32
F32R = mybir.dt.float32r
AF = mybir.ActivationFunctionType
H16 = np.float16

B, S, D = 2, 2048, 1024
H, DK = 16, 64
HPC = 4                # heads per core
N_CORES = 8
THETA = 10000.0
SCALE = 1.0 / (DK ** 0.5)
CSHIFT = 3.0           # exp shift keeps weights in fp16's sweet spot

NS = S // 512          # 4 q-ranges of 512 (= x col-chunks "nch")
SB = S // 128          # 16 k-blocks of 128


# ---------------------------------------------------------------------------
# Device program (identical on all 8 cores; only the input shards differ)
# ---------------------------------------------------------------------------
def _build_program():
    nc = bacc.Bacc("TRN2", target_bir_lowering=False, debug=False,
                   num_devices=N_CORES)

    xA = nc.dram_tensor("xA", [128, 8 * S], F16, kind="ExternalInput")
    wqeA = nc.dram_tensor("wqeA", [128, 1024], F16, kind="ExternalInput")
    wqoA = nc.dram_tensor("wqoA", [128, 1024], F16, kind="ExternalInput")
    wkeA = nc.dram_tensor("wkeA", [128, 1024], F16, kind="ExternalInput")
    wkoA = nc.dram_tensor("wkoA", [128, 1024], F16, kind="ExternalInput")
    wvA = nc.dram_tensor("wvA", [128, 2048], F16, kind="ExternalInput")
    woA = nc.dram_tensor("woA", [128, 2048], F16, kind="ExternalInput")
    csA = nc.dram_tensor("csA", [128, 2 * S], F16, kind="ExternalInput")
    # [tri(128) | zeros+tri(256)]
    mkA = nc.dram_tensor("mkA", [128, 384], F16, kind="ExternalInput")
    onesT = nc.dram_tensor("onesT", [1, 64], F32R, kind="ExternalInput")
    out = nc.dram_tensor("out", [S, D], F32, kind="ExternalOutput")

    with tile.TileContext(nc) as tc, ExitStack() as ctx:
        cons = ctx.enter_context(tc.tile_pool(name="cons", bufs=1))

        # ---- persistent SBUF tensors -----------------------------------
        xa = cons.tile([128, 8 * S], F16, tag="xa", name="xa")
        wqe = cons.tile([128, 1024], F16, tag="wqe", name="wqe")
        wqo = cons.tile([128, 1024], F16, tag="wqo", name="wqo")
        wke = cons.tile([128, 1024], F16, tag="wke", name="wke")
        wko = cons.tile([128, 1024], F16, tag="wko", name="wko")
        wv = cons.tile([128, 2048], F16, tag="wv", name="wv")
        wo = cons.tile([128, 2048], F16, tag="wo", name="wo")
        csa = cons.tile([128, 2 * S], F16, tag="csa", name="csa")
        cos_t, sin_t = csa[:, 0:S], csa[:, S:2 * S]
        mka = cons.tile([128, 384], F16, tag="mka", name="mka")
        tri_m, zt_m = mka[:, 0:128], mka[:, 128:384]
        # q/k fp8 in DoubleRow score layout: head h on partition rows
        # 32h..32h+31; q: subtile i at cols i*S + s; k kb-interleaved:
        # cols 256*kb + 128*i + c (contiguous [p,2,128] ldweights slices)
        qc2 = cons.tile([128, 2 * S], FP8, tag="qc2", name="qc2")
        kc2 = cons.tile([128, 2 * S], FP8, tag="kc2", name="kc2")
        # V per k-block kb, head h: cols 260*kb + 65*h + f; f==64 is the
        # ones column (denominator accumulator).
        v2t = cons.tile([128, SB * 260], F16, tag="v2t", name="v2t")
        # normalized O^T in outproj layout: cols 256*s + 128*i + c
        otb2 = cons.tile([128, 2 * S], F16, tag="otb2", name="otb2")
        bias_t = cons.tile([128, 1], F32, tag="bias", name="bias")
        ones64 = cons.tile([1, 64], F32R, tag="on64", name="on64")

        nc.gpsimd.memset(bias_t[:], -CSHIFT)
        nc.sync.dma_start(ones64[:], onesT[:, :])
        nc.gpsimd.memset(v2t[:], 1.0)

        # input DMAs, ordered by first use
        nc.sync.dma_start(wqe[:], wqeA[:, :])
        nc.sync.dma_start(wqo[:], wqoA[:, :])
        nc.sync.dma_start(wke[:], wkeA[:, :])
        nc.sync.dma_start(wko[:], wkoA[:, :])
        for n in range(NS):
            nc.sync.dma_start(
                xa[:].rearrange("p (ch s) -> p ch s", s=S)
                     [:, :, 512 * n:512 * (n + 1)],
                xA[:, :].rearrange("p (ch s) -> p ch s", s=S)
                        [:, :, 512 * n:512 * (n + 1)])
            nc.sync.dma_start(
                csa[:].rearrange("p (two s) -> p two s", two=2)
                      [:, :, 512 * n:512 * (n + 1)],
                csA[:, :].rearrange("p (two s) -> p two s", two=2)
                         [:, :, 512 * n:512 * (n + 1)])
            if n == 0:
                nc.sync.dma_start(wv[:], wvA[:, :])
                nc.sync.dma_start(mka[:], mkA[:, :])
            elif n == 1:
                nc.sync.dma_start(wo[:], woA[:, :])

        psum = ctx.enter_context(tc.tile_pool(name="psum", bufs=2,
                                              space="PSUM"))
        stg = ctx.enter_context(tc.tile_pool(name="stg", bufs=2))
        atp = ctx.enter_context(tc.tile_pool(name="atp", bufs=4))
        rp = ctx.enter_context(tc.tile_pool(name="rope", bufs=6))
        rcp = ctx.enter_context(tc.tile_pool(name="rcp", bufs=3))
        obp = ctx.enter_context(tc.tile_pool(name="obp", bufs=4))

        stage = {}

        # ---- building blocks -------------------------------------------
        def qk_group(wt, key, n):
            # one projected 128-feature tile over q-range n
            ps = psum.tile([128, 512], F32, tag="aux", name="qkg", bufs=2)
            for ch in range(8):
                nc.tensor.matmul(
                    ps[:], wt[:, 128 * ch:128 * (ch + 1)],
                    xa[:, S * ch + 512 * n:S * ch + 512 * (n + 1)],
                    start=(ch == 0), stop=(ch == 7),
                    skip_group_check=True)
            st = stg.tile([128, 512], F16, tag=key, name=key, bufs=3)
            nc.vector.tensor_copy(st[:], ps[:])
            stage[(key, n)] = st

        def v_group(s):
            # V for k-block s (8 accumulating matmuls)
            ps = psum.tile([128, 512], F32, tag="aux", name="vg", bufs=2)
            for ch in range(8):
                nc.tensor.matmul(
                    ps[:, 0:256],
                    xa[:, S * ch + 128 * s:S * ch + 128 * (s + 1)],
                    wv[:, 256 * ch:256 * (ch + 1)],
                    start=(ch == 0), stop=(ch == 7),
                    skip_group_check=True)
            dst = (v2t[:, 260 * s:260 * (s + 1)]
                   .rearrange("p (h f) -> p h f", f=65)[:, :, 0:64])
            nc.vector.tensor_copy(
                dst, ps[:, 0:256].rearrange("p (h f) -> p h f", f=64))

        def rope(which, n):
            qe = stage.pop((which + "e", n))
            qo = stage.pop((which + "o", n))
            sl = bass.ts(n, 512)
            mul = nc.vector.tensor_mul if n == 0 else nc.gpsimd.tensor_mul
            if which == "q":
                d_re = qc2[:, 0:S][:, sl]
                d_ro = qc2[:, S:2 * S][:, sl]
            else:
                v = (kc2[:, 1024 * n:1024 * (n + 1)]
                     .rearrange("p (kb two c) -> p kb two c", two=2, c=128))
                d_re, d_ro = v[:, :, 0, :], v[:, :, 1, :]
            t1 = rp.tile([128, 512], F16, tag="rt", name="t1")
            mul(t1[:], qe[:], cos_t[:, sl])
            t2 = rp.tile([128, 512], F16, tag="rt", name="t2")
            mul(t2[:], qo[:], sin_t[:, sl])
            nc.vector.tensor_sub(d_re, t1[:], t2[:])
            t3 = rp.tile([128, 512], F16, tag="rt", name="t3")
            mul(t3[:], qe[:], sin_t[:, sl])
            t4 = rp.tile([128, 512], F16, tag="rt", name="t4")
            mul(t4[:], qo[:], cos_t[:, sl])
            nc.vector.tensor_add(d_ro, t3[:], t4[:])

        def scores(qr, h, p, scs):
            # kb pair (2p, 2p+1) for head h into 2-bank pair tile scs
            off = max(0, 256 * p - 512 * qr)
            q0 = 512 * qr
            qv = (qc2[32 * h:32 * h + 32, :]
                  .rearrange("p (two s) -> p two s", two=2)
                  [:, :, q0 + off:q0 + 512])
            for half in range(2):
                kb = 2 * p + half
                nc.tensor.matmul(
                    scs[:, 512 * half + off:512 * half + 512],
                    kc2[32 * h:32 * h + 32, 256 * kb:256 * (kb + 1)]
                       .rearrange("p (two m) -> p two m", two=2),
                    qv, start=True, stop=True, perf_mode=DR,
                    tile_position=(32 * h, 0), skip_group_check=True)

        def expv(qr, h, p, scs):
            off = max(0, 256 * p - 512 * qr)
            at = atp.tile([128, 1024], F16, tag="at2", name="at2")
            nc.scalar.activation(
                at[:].rearrange("p (two c) -> p two c", two=2)[:, :, off:512],
                scs[:].rearrange("p (two c) -> p two c", two=2)[:, :, off:512],
                AF.Exp, scale=SCALE, bias=bias_t[:])
            if p >= 2 * qr:  # diagonal pair: causal masks on Pool
                nc.gpsimd.tensor_mul(at[:, off:off + 128],
                                     at[:, off:off + 128], tri_m[:])
                nc.gpsimd.tensor_mul(at[:, 512 + off:512 + off + 256],
                                     at[:, 512 + off:512 + off + 256],
                                     zt_m[:])
            return at

        def attn_v(qr, h, p, at, ops):
            off = max(0, 256 * p - 512 * qr)
            nkb = 4 * (qr + 1)
            for half in range(2):
                kb = 2 * p + half
                nc.tensor.matmul(
                    ops[0:65, off:512],
                    v2t[:, 260 * kb + 65 * h:260 * kb + 65 * (h + 1)],
                    at[:, 512 * half + off:512 * (half + 1)],
                    start=(kb == 0), stop=(kb == nkb - 1),
                    skip_group_check=True)

        def otb_slice(qr, h):
            r0 = 64 * (h % 2)
            return (otb2[r0:r0 + 64, :]
                    .rearrange("p (s two c) -> p s two c", two=2, c=128)
                    [:, 4 * qr:4 * qr + 4, h // 2, :])

        def divide_pair(qr, heads, opss, final=False):
            rcs, bcs = {}, {}
            if final:
                # tail path: PSUM->SBUF copy + PE fp32r broadcast
                ovs = {}
                for h in heads:
                    ovs[h] = rcp.tile([65, 512], F32, tag=f"o65{h % 2}",
                                      name="o65")
                    nc.vector.tensor_copy(ovs[h][:], opss[h][0:65, :])
                for h in heads:
                    rcs[h] = rcp.tile([1, 512], F32R, tag=f"rc{h % 2}",
                                      name="rc")
                    with nc.allow_low_precision("fp32r recip broadcast"):
                        nc.vector.reciprocal(rcs[h][:], ovs[h][64:65, :])
                for h in heads:
                    bc = psum.tile([128, 512], F32, tag="aux", name="bc",
                                   bufs=2)
                    nc.tensor.matmul(bc[0:64, :], ones64[:], rcs[h][:],
                                     start=True, stop=True)
                    bcs[h] = bc
                for h in heads:
                    nc.vector.tensor_mul(otb_slice(qr, h), ovs[h][0:64, :],
                                         bcs[h][0:64, :])
                return
            # steady-state: recip from PSUM, partition-broadcast via
            # SP-queue SBUF->SBUF DMA, single-PSUM-operand multiply
            for h in heads:
                rcs[h] = rcp.tile([1, 512], F32R, tag=f"rc{h % 2}",
                                  name="rc")
                with nc.allow_low_precision("fp32r recip broadcast"):
                    nc.vector.reciprocal(rcs[h][:], opss[h][64:65, :])
            for h in heads:
                rcb = rcp.tile([64, 512], F32R, tag=f"rcb{h % 2}",
                               name="rcb")
                nc.sync.dma_start(
                    rcb[:],
                    rcs[h][:].rearrange("p (one c) -> p one c", one=1)
                             .to_broadcast((1, 64, 512)))
                bcs[h] = rcb
            for h in heads:
                nc.vector.tensor_mul(otb_slice(qr, h), opss[h][0:64, :],
                                     bcs[h][:])

        def outproj(qr):
            for s in range(4 * qr, 4 * (qr + 1)):
                for hd in range(2):
                    ps = psum.tile([128, 512], F32, tag="aux", name="op",
                                   bufs=2)
                    for i in range(2):
                        nc.tensor.matmul(
                            ps[:],
                            otb2[:, 256 * s + 128 * i:
                                 256 * s + 128 * (i + 1)],
                            wo[:, 1024 * hd + 512 * i:
                               1024 * hd + 512 * (i + 1)],
                            start=(i == 0), stop=(i == 1),
                            skip_group_check=True)
                    ob = obp.tile([128, 512], F32, tag="ob", name="ob")
                    if qr == NS - 1:
                        nc.scalar.copy(ob[:], ps[:])
                    else:
                        nc.vector.tensor_copy(ob[:], ps[:])
                    nc.sync.dma_start(
                        out[128 * s:128 * (s + 1),
                            512 * hd:512 * (hd + 1)], ob[:])

        # ---- stage A: everything attention(qr=0) needs -----------------
        for key, wt in (("qe", wqe), ("qo", wqo)):
            qk_group(wt, key, 0)
        rope("q", 0)
        for key, wt in (("ke", wke), ("ko", wko)):
            qk_group(wt, key, 0)
        rope("k", 0)
        for s in range(4):
            v_group(s)

        # remaining projection work as filler between pair-steps
        fillers = []
        for n in range(1, NS):
            fillers.append((n, lambda n=n: qk_group(wqe, "qe", n)))
            fillers.append((n, lambda n=n: qk_group(wqo, "qo", n)))
            fillers.append((n, lambda n=n: rope("q", n)))
            fillers.append((n, lambda n=n: qk_group(wke, "ke", n)))
            fillers.append((n, lambda n=n: qk_group(wko, "ko", n)))
            fillers.append((n, lambda n=n: rope("k", n)))
            for s in range(4 * n, 4 * (n + 1)):
                fillers.append((n, lambda s=s: v_group(s)))

        def pop_filler(ready_for_qr=None, step_qr=None):
            if ready_for_qr is not None:
                while fillers and fillers[0][0] <= ready_for_qr:
                    fillers.pop(0)[1]()
                return
            for _ in range(3):
                if fillers and fillers[0][0] <= step_qr + 1:
                    fillers.pop(0)[1]()

        # ---- attention, software-pipelined -----------------------------
        pending = [None]

        def flush_pending(final=False):
            if pending[0] is None:
                return
            qr, hp, p, ats, opss, last = pending[0]
            for h in (2 * hp, 2 * hp + 1):
                attn_v(qr, h, p, ats[h], opss[h])
            if last:
                divide_pair(qr, (2 * hp, 2 * hp + 1), opss,
                            final=(final and qr == NS - 1 and hp == 1))
                if hp == 1:
                    outproj(qr)
            pending[0] = None

        for qr in range(NS):
            pop_filler(ready_for_qr=qr)
            for hp in range(2):
                heads = (2 * hp, 2 * hp + 1)
                npair = 2 * (qr + 1)
                opss = {h: psum.tile([128, 512], F32, tag=f"ops{h % 2}",
                                     name=f"ops{h % 2}", bufs=1)
                        for h in heads}
                for p in range(npair):
                    scss = {h: psum.tile([128, 1024], F32, tag=f"sc{h % 2}",
                                         name=f"sc{h % 2}", bufs=1)
                            for h in heads}
                    for h in heads:
                        scores(qr, h, p, scss[h])
                    flush_pending()
                    ats = {h: expv(qr, h, p, scss[h]) for h in heads}
                    pending[0] = (qr, hp, p, ats, opss, p == npair - 1)
                    pop_filler(step_qr=qr)
        flush_pending(final=True)

    if not nc.is_finalized():
        nc.finalize()
    return nc


_CACHE = {}


def _get_nc():
    if "nc" not in _CACHE:
        _CACHE["nc"] = _build_program()
    return _CACHE["nc"]


# ---------------------------------------------------------------------------
# Host side: shard, run, gather
# ---------------------------------------------------------------------------
def _shared_consts(token_positions):
    pos = np.asarray(token_positions).astype(np.float32)
    inv = THETA ** (-np.arange(0, DK, 2, dtype=np.float32) / DK)  # [32]
    ang = pos[:, None] * inv[None, :]                             # [S, 32]
    cosT = np.tile(np.cos(ang).T, (4, 1))                         # [128, S]
    sinT = np.tile(np.sin(ang).T, (4, 1))
    csA = np.ascontiguousarray(
        np.concatenate([cosT, sinT], axis=1)).astype(H16)

    kk, cc = np.meshgrid(np.arange(128), np.arange(128), indexing="ij")
    tri = (cc >= kk).astype(np.float32)                # valid: q >= k
    kk2, cc2 = np.meshgrid(np.arange(128), np.arange(256), indexing="ij")
    zt = np.where(cc2 < 128, 0.0,
                  (cc2 - 128 >= kk2).astype(np.float32))
    mkA = np.ascontiguousarray(
        np.concatenate([tri, zt], axis=1)).astype(H16)
    return csA, mkA


def _core_inputs(c, x, Wq, Wk, Wv, Wo, csA, mkA):
    b, hg = c // 4, c % 4
    xT = np.ascontiguousarray(x[b].T)  # [D, S]
    xA = np.ascontiguousarray(
        xT.reshape(8, 128, S).transpose(1, 0, 2).reshape(128, 8 * S)
    ).astype(H16)

    m = np.arange(128)

    def pack_qk(W, parity):
        rows = 64 * (4 * hg + m // 32) + 2 * (m % 32) + parity
        t = W[rows, :].T.reshape(8, 128, 128)          # [ch, p, m]
        return np.ascontiguousarray(
            t.transpose(1, 0, 2).reshape(128, 1024)).astype(H16)

    f = np.arange(256)
    rows_v = 64 * (4 * hg + f // 64) + (f % 64)
    tv = Wv[rows_v, :].T.reshape(8, 128, 256)          # [ch, p, f]
    wvA = np.ascontiguousarray(
        tv.transpose(1, 0, 2).reshape(128, 2048)).astype(H16)

    wo_i = []
    for i in range(2):
        vcols = 64 * (4 * hg + 2 * i + m // 64) + (m % 64)
        wo_i.append(Wo[:, vcols].T)                     # [128 p, 1024 d]
    arr = np.stack(wo_i, axis=1)                        # [p, i, d]
    woA = np.ascontiguousarray(
        arr.reshape(128, 2, 2, 512).transpose(0, 2, 1, 3).reshape(128, 2048)
    ).astype(H16)

    return {
        "xA": xA,
        "wqeA": pack_qk(Wq, 0), "wqoA": pack_qk(Wq, 1),
        "wkeA": pack_qk(Wk, 0), "wkoA": pack_qk(Wk, 1),
        "wvA": wvA, "woA": woA,
        "csA": csA, "mkA": mkA,
        "onesT": np.ones((1, 64), np.float32),
    }


def _run(x, Wq, Wk, Wv, Wo, token_positions, **spmd_kwargs):
    x = np.asarray(x, np.float32)
    Wq = np.asarray(Wq, np.float32)
    Wk = np.asarray(Wk, np.float32)
    Wv = np.asarray(Wv, np.float32)
    Wo = np.asarray(Wo, np.float32)

    csA, mkA = _shared_consts(token_positions)
    in_maps = [_core_inputs(c, x, Wq, Wk, Wv, Wo, csA, mkA)
               for c in range(N_CORES)]
    res = run_bass_kernel_spmd(_get_nc(), in_maps,
                               core_ids=list(range(N_CORES)), **spmd_kwargs)
    outf = np.zeros((B, S, D), np.float32)
    for c in range(N_CORES):
        outf[c // 4] += res.results[c]["out"]
    return outf, res


def kernel(x, Wq, Wk, Wv, Wo, token_positions):
    outf, _ = _run(x, Wq, Wk, Wv, Wo, token_positions)
    return outf
